# revision 10
# baseline (speedup 1.0000x reference)
"""Fused single-head attention (projections + softmax attention) on 8 TRN2
NeuronCores.

Problem: B=4, S=4096, H=1024, D=64
  q = query @ Wq + bq ; k = key @ Wk + bk ; v = value @ Wv + bv
  out = softmax(q k^T / sqrt(D), mask over k) @ v

Sharding: core c -> (batch b = c//2, query half h = c%2). Each core
computes 2048 queries against the batch's keys. No collectives.

Key-compaction: masked keys contribute exactly zero to both the softmax
numerator and denominator (reference maps them to exp(-1e9) == 0 in
f32), so the host gathers only the unmasked keys/values per batch
(~2048 of 4096) and zero-pads to a 128-column multiple KC = kptc*128.
All device work (DMA, projections, scores, exp, attv) is clamped to KC.

Contraction padding: on this silicon a bf16 matmul streams its moving
operand at ~0.42 ns/col when the contraction (partition) dim is 128,
but ~0.83 ns/col when it is 64 (measured; independent of dtype and of
the stationary free dim). The head dim D=64 would put every score
matmul on the slow path, so the host pads Wq/Wk/Wv and biases to 128
output dims (rows 64..127 all zero). Projections then produce
{q,k,v}_projT as [128, seq] tiles whose lower half is zero, and the
score matmuls contract over the full 128 partitions at the fast rate
for identical math.

DMA-issue economy: a HWDGE dma_start with a scattered source AP costs
~1-2 us on the issuing engine (descriptor generation), which made the
input feed issue-bound. The host therefore pre-packs every input chunk
as its own [128, HT, cw] array (contiguous per partition), so each
chunk is ONE cheap dma_start. Weights are pre-packed [128, HT, 128]
the same way.

Pipeline weave: emission interleaves attv tile-ranges of chunk c-1
between the score/exp pairs of chunk c, which rate-matches the
in-order PE (~1.1 us per woven group) against the scalar exp stream
(~1.05 us per pair), keeping both engines fed. The scalar engine's
only non-exp work is two head DMAs that complete before the first exp.
The full teardown (clear_and_free_semaphores' dma_reset) is kept: it
fences in-flight output DMAs. kernel() runs an untraced warm-up
execution first — the first execution of a freshly loaded NEFF can
return corrupted data.
"""

import ml_dtypes
import numpy as np

import concourse.bass as bass
import concourse.mybir as mybir
import concourse.tile as tile
from concourse.masks import make_identity
from concourse.vector_clock import ScopedClock

B, S, H, D = 4, 4096, 1024, 64
DP = 128             # zero-padded head dim (contraction fast path)
NCORES = 8
SQ = S // 2          # queries per core
HT = H // 128        # 8 contraction chunks
QCH = 512            # matmul moving free dim

FP = mybir.dt.float32
BF = mybir.dt.bfloat16

# ---------------------------------------------------------------------------
# Walrus in this container rejects >1 sync-wait per instruction; peel extra
# waits onto same-engine nops (engine streams are in-order).
_orig_commit = tile.TileContext._commit_instruction


def _split_waits(self, inst):
    si = inst.sync_info
    if si is None or not si.on_wait or len(si.on_wait) <= 1:
        return
    waits = list(si.on_wait)
    si.on_wait = waits[-1:]
    for w in waits[:-1]:
        nop = mybir.InstNoOp(
            name=self.nc.get_next_instruction_name(),
            sync_info=mybir.SyncInfo(on_wait=[w], on_update=[]),
            bass_nofuse=True,
            engine=inst.engine,
            ins=[],
            outs=[],
        )
        _orig_commit(self, nop)


def _patched_commit(self, inst, lazy_reg_writes=True):
    _split_waits(self, inst)
    return _orig_commit(self, inst, lazy_reg_writes)


def _patched_drain_and_barrier(self, tick_clock, wait_clock):
    # Keep the full teardown: clear_and_free_semaphores' dma_reset is what
    # fences in-flight output DMAs before the NEFF ends — removing it made
    # results flaky. Only the >1-wait splitting differs from stock tile.
    nc = self.nc
    collector = nc.sync.nop(nofuse=True, hint="tile_drain_waits")
    wait_clock.add_sem_waits(
        collector.ins, ScopedClock({None: tick_clock.global_clock})
    )
    si = collector.ins.sync_info
    if si is not None and si.on_wait and len(si.on_wait) > 1:
        waits = list(si.on_wait)
        si.on_wait = waits[:1]
        for w in waits[1:]:
            extra = nc.sync.nop(nofuse=True, hint="tile_drain_waits")
            if extra.ins.sync_info is None:
                extra.ins.sync_info = mybir.SyncInfo(on_wait=[w], on_update=[])
            else:
                extra.ins.sync_info.on_wait = [w]
    nc.sync.drain()
    nc.all_engine_barrier()
    assert self.sems is not None
    popped = nc._tile_sem_poison_stack.pop()
    assert popped is self._sem_poison
    nc.clear_and_free_semaphores(list(self.sems.allocated().values()))
    nc.all_engine_barrier()


tile.TileContext._commit_instruction = _patched_commit
tile.TileContext._drain_and_barrier = _patched_drain_and_barrier
# ---------------------------------------------------------------------------

AF = mybir.ActivationFunctionType


def _chunks(kptc):
    KC = 128 * kptc
    kch = [256, 1024]
    while sum(kch) + 1024 <= KC:
        kch.append(1024)
    if sum(kch) < KC:
        kch.append(KC - sum(kch))
    qch = [512, 512, 1024]
    vch = [1024] * (KC // 1024)
    if KC % 1024:
        vch.append(KC % 1024)
    return kch, qch, vch


def _build(kptc):
    """Build for kptc projected key tiles (KC = 128*kptc compacted cols)."""
    KC = 128 * kptc
    kch, qch, vch = _chunks(kptc)

    nc = bass.Bass(trn_type="TRN2")

    def dp(name, shape, dt=BF):
        return nc.declare_dram_parameter(name, shape, dt, isOutput=False)

    kx = [dp(f"kx{i}", [128, HT, cw]) for i, cw in enumerate(kch)]
    qx = [dp(f"qx{i}", [128, HT, cw]) for i, cw in enumerate(qch)]
    vx = [dp(f"vx{i}", [128, HT, cw]) for i, cw in enumerate(vch)]
    maskT = dp("maskT", [128, kptc], FP)
    wq = dp("wq", [128, HT, DP])
    wk = dp("wk", [128, HT, DP])
    wv = dp("wv", [128, HT, DP])
    bq = dp("bq", [DP, 1], FP)
    bk = dp("bk", [DP, 1], FP)
    bv = dp("bv", [DP, 1], FP)
    # Row D holds the softmax denominator; the host divides + transposes.
    outT = nc.declare_dram_parameter("outT", [D + 1, SQ], FP, isOutput=True)

    with tile.TileContext(nc) as tc:
        with (
            tc.tile_pool(name="const", bufs=1) as cpool,
            tc.tile_pool(name="proj", bufs=1) as projpool,
            tc.tile_pool(name="xin", bufs=4) as xpool,
            tc.tile_pool(name="expb", bufs=3) as exppool,
            tc.tile_pool(name="outs", bufs=1) as outpool,
            tc.tile_pool(name="big", bufs=3, space="PSUM") as ps_big,
            tc.tile_pool(name="att", bufs=2, space="PSUM") as ps_att,
        ):
            # ---- constants ------------------------------------------------
            wq_s = cpool.tile([128, HT, DP], BF, tag="wq")
            wk_s = cpool.tile([128, HT, DP], BF, tag="wk")
            wv_s = cpool.tile([128, HT, DP], BF, tag="wv")
            nc.scalar.dma_start(wk_s[:], wk[:, :, :])
            nc.sync.dma_start(wq_s[:], wq[:, :, :])
            bq_s = cpool.tile([DP, 1], FP, tag="bq")
            bk_s = cpool.tile([DP, 1], FP, tag="bk")
            bv_s = cpool.tile([DP, 1], FP, tag="bv")
            maskT_s = cpool.tile([128, kptc], FP, tag="mask")

            # ---- projections: {q,k,v}_projT [128, seq] bf16 ---------------
            # rows 64..127 are exact zeros (padded weights+biases) so score
            # matmuls can contract over 128 partitions at the fast rate.
            q_projT = projpool.tile([DP, SQ], BF, tag="qproj")
            k_projT = projpool.tile([DP, KC], BF, tag="kproj")
            v_projT = projpool.tile([DP, KC], BF, tag="vproj")

            def proj_chunk(nm, dst, src, w_s, b_s, c0, cw, eng):
                xt = xpool.tile(
                    [128, HT, 1024], BF, tag="xin", name=f"x{nm}{c0}"
                )
                eng.dma_start(xt[:, :, :cw], src[:, :, :])
                ps = ps_big.tile([128, 1024], FP, tag="big", name=f"ps{nm}{c0}")
                for j in range(0, cw, QCH):
                    jw = min(QCH, cw - j)
                    for o in range(HT):
                        nc.tensor.matmul(
                            ps[:, j : j + jw],
                            w_s[:, o, :],
                            xt[:, o, j : j + jw],
                            start=(o == 0),
                            stop=(o == HT - 1),
                        )
                nc.vector.tensor_scalar_add(
                    dst[:, c0 : c0 + cw], ps[:, :cw], b_s[:, :]
                )

            koff = [0]
            for cw in kch:
                koff.append(koff[-1] + cw)
            qoff = [0]
            for cw in qch:
                qoff.append(qoff[-1] + cw)
            voff = [0]
            for cw in vch:
                voff.append(voff[-1] + cw)

            def k_chunk(i, eng):
                proj_chunk("k", k_projT, kx[i], wk_s, bk_s, koff[i], kch[i], eng)

            def q_chunk(i, eng):
                proj_chunk("q", q_projT, qx[i], wq_s, bq_s, qoff[i], qch[i], eng)

            def v_chunk(i, eng):
                proj_chunk("v", v_projT, vx[i], wv_s, bv_s, voff[i], vch[i], eng)

            ident = cpool.tile([128, 128], BF, tag="ident")
            make_identity(nc, ident[:])

            # ---- v_aug [128, kptc, 65] bf16 = [v*m | m] -------------------
            v_aug = projpool.tile([128, kptc, D + 1], BF, tag="vaug")

            def v_trans(lo, hi):
                for t in range(lo, hi):
                    tp = ps_big.tile(
                        [128, 1024], BF, tag="big", name=f"tp{t}"
                    )
                    nc.tensor.transpose(
                        tp[:, :D],
                        v_projT[:, t * 128 : (t + 1) * 128],
                        ident[:, :D],
                    )
                    nc.vector.tensor_scalar_mul(
                        v_aug[:, t, :D], tp[:, :D], maskT_s[:, t : t + 1]
                    )
                    nc.vector.tensor_copy(
                        v_aug[:, t, D : D + 1], maskT_s[:, t : t + 1]
                    )

            # ---- attention, software-pipelined over query chunks ----------
            outT_s = outpool.tile([D + 1, SQ], FP, tag="outT")
            exp_tiles = {}

            def scores_part(c, lo, hi):
                if c not in exp_tiles:
                    exp_tiles[c] = exppool.tile(
                        [128, kptc, QCH], BF, tag="expT", name=f"expT{c}"
                    )
                expTc = exp_tiles[c]
                q0 = c * QCH
                t = lo
                while t < hi:
                    tw = min(2, hi - t)
                    sp = ps_big.tile(
                        [128, 1024], FP, tag="big", name=f"sp{c}_{t}"
                    )
                    for j in range(tw):
                        nc.tensor.matmul(
                            sp[:, j * QCH : (j + 1) * QCH],
                            k_projT[:, (t + j) * 128 : (t + j + 1) * 128],
                            q_projT[:, q0 : q0 + QCH],
                            start=True,
                            stop=True,
                        )
                    nc.scalar.activation(
                        expTc[:, t : t + tw, :],
                        sp[:, : tw * QCH],
                        AF.Exp,
                        scale=0.125,
                    )
                    t += tw

            att_ps = {}

            def attv_part(c, lo, hi):
                if c not in att_ps:
                    att_ps[c] = ps_att.tile(
                        [128, QCH], FP, tag="att", name=f"att{c}"
                    )
                ap = att_ps[c]
                expTc = exp_tiles[c]
                for t in range(lo, hi):
                    nc.tensor.matmul(
                        ap[: D + 1, :],
                        v_aug[:, t, :],
                        expTc[:, t, :],
                        start=(t == 0),
                        stop=(t == kptc - 1),
                    )

            def attv_fin(c):
                ap = att_ps.pop(c)
                exp_tiles.pop(c)
                nc.vector.tensor_copy(
                    outT_s[:, c * QCH : (c + 1) * QCH], ap[: D + 1, :]
                )
                eng = nc.gpsimd if c % 2 == 0 else nc.sync
                eng.dma_start(
                    outT[:, c * QCH : (c + 1) * QCH],
                    outT_s[:, c * QCH : (c + 1) * QCH],
                )

            def ktiles(i):
                return (koff[i] // 128, koff[i + 1] // 128)

            T1 = (kptc // 3) & ~1
            T2 = ((2 * kptc) // 3) & ~1

            # ---- emission, sorted by data arrival -------------------------
            k_chunk(0, nc.scalar)
            q_chunk(0, nc.sync)
            nc.gpsimd.dma_start(bk_s[:], bk[:, :])
            nc.gpsimd.dma_start(bq_s[:], bq[:, :])
            nc.gpsimd.dma_start(wv_s[:], wv[:, :, :])
            nc.gpsimd.dma_start(bv_s[:], bv[:, :])
            nc.gpsimd.dma_start(maskT_s[:], maskT[:, :])
            scores_part(0, *ktiles(0))
            k_chunk(1, nc.gpsimd)
            q_chunk(1, nc.sync)
            scores_part(0, *ktiles(1))
            k_chunk(2, nc.gpsimd)
            q_chunk(2, nc.sync)
            scores_part(0, *ktiles(2))
            if len(kch) > 3:
                k_chunk(3, nc.gpsimd)
                scores_part(0, *ktiles(3))
            scores_part(1, 0, T1)
            v_chunk(0, nc.gpsimd)
            scores_part(1, T1, T2)
            v_trans(0, 8)
            scores_part(1, T2, kptc)
            v_chunk(1, nc.sync)
            scores_part(2, 0, T1)
            v_trans(8, min(16, kptc))
            attv_part(0, 0, T1)
            scores_part(2, T1, T2)
            attv_part(0, T1, T2)
            for i in range(2, len(vch)):
                v_chunk(i, nc.sync)
            if kptc > 16:
                v_trans(16, kptc)
            scores_part(2, T2, kptc)
            attv_part(0, T2, kptc)
            attv_fin(0)
            scores_part(3, 0, T1)
            attv_part(1, 0, T1)
            scores_part(3, T1, T2)
            attv_part(1, T1, T2)
            scores_part(3, T2, kptc)
            attv_part(1, T2, kptc)
            attv_fin(1)
            attv_part(2, 0, kptc)
            attv_fin(2)
            attv_part(3, 0, kptc)
            attv_fin(3)

    return nc


_NC_CACHE = {}
LAST_RESULT = None


def kernel(query, key, value, mask, Wq, bq, Wk, bk, Wv, bv):
    global LAST_RESULT
    import os

    from concourse.bass_utils import run_bass_kernel_spmd

    bf16 = ml_dtypes.bfloat16
    query = np.asarray(query, np.float32)
    key = np.asarray(key, np.float32)
    value = np.asarray(value, np.float32)
    maskf = np.asarray(mask).astype(np.float32)

    def padw(w):
        # [H, D] -> packed [128, HT, DP] bf16 with output dims 64..127 zero
        wp = np.zeros((H, DP), np.float32)
        wp[:, :D] = np.asarray(w, np.float32)
        return np.ascontiguousarray(
            wp.reshape(HT, 128, DP).transpose(1, 0, 2)
        ).astype(bf16)

    def padb(b):
        bp = np.zeros((DP, 1), np.float32)
        bp[:D, 0] = np.asarray(b, np.float32)
        return bp

    def pack_chunks(x, widths):
        # x [cols, H] f32 -> list of [128, HT, cw] bf16 (contiguous)
        xt = x.T.astype(bf16)  # [H, cols]
        out = []
        c = 0
        for cw in widths:
            out.append(
                np.ascontiguousarray(
                    xt[:, c : c + cw].reshape(HT, 128, cw).transpose(1, 0, 2)
                )
            )
            c += cw
        return out

    Wqb, Wkb, Wvb = padw(Wq), padw(Wk), padw(Wv)
    bqp, bkp, bvp = padb(bq), padb(bk), padb(bv)

    # Key compaction: keep only unmasked keys, pad to a 128 multiple.
    idx = [np.nonzero(maskf[b])[0] for b in range(B)]
    maxk = max(len(i) for i in idx)
    kptc = max(4, (maxk + 127) // 128)
    KC = 128 * kptc
    kch, qch, vch = _chunks(kptc)

    in_maps = []
    for c in range(NCORES):
        b, h = divmod(c, 2)
        qs = slice(h * SQ, (h + 1) * SQ)
        ki = idx[b]
        kc = np.zeros((KC, H), np.float32)
        vc = np.zeros((KC, H), np.float32)
        kc[: len(ki)] = key[b][ki]
        vc[: len(ki)] = value[b][ki]
        mc = np.zeros(KC, np.float32)
        mc[: len(ki)] = 1.0
        im = {
            "maskT": np.ascontiguousarray(mc.reshape(kptc, 128).T),
            "wq": Wqb,
            "wk": Wkb,
            "wv": Wvb,
            "bq": bqp,
            "bk": bkp,
            "bv": bvp,
        }
        for i, a in enumerate(pack_chunks(kc, kch)):
            im[f"kx{i}"] = a
        for i, a in enumerate(pack_chunks(query[b, qs], qch)):
            im[f"qx{i}"] = a
        for i, a in enumerate(pack_chunks(vc, vch)):
            im[f"vx{i}"] = a
        in_maps.append(im)

    if kptc not in _NC_CACHE:
        _NC_CACHE[kptc] = _build(kptc)

    # Warm-up execution (untraced): the very first execution of a freshly
    # loaded NEFF can race engine/DGE warm-up and return corrupted data;
    # the second execution is deterministic. Results come from the real run.
    os.environ["BASS_NEVER_TRACE"] = "1"
    try:
        run_bass_kernel_spmd(
            _NC_CACHE[kptc], in_maps, core_ids=list(range(NCORES))
        )
    finally:
        del os.environ["BASS_NEVER_TRACE"]

    res = run_bass_kernel_spmd(
        _NC_CACHE[kptc], in_maps, core_ids=list(range(NCORES))
    )
    LAST_RESULT = res

    outv = np.empty((B, S, D), np.float32)
    for c in range(NCORES):
        b, h = divmod(c, 2)
        r = res.results[c]["outT"]  # [D+1, SQ]: numerator rows + denominator
        outv[b, h * SQ : (h + 1) * SQ] = (r[:D] / r[D : D + 1]).T
    return outv
